# revision 19
# baseline (speedup 1.0000x reference)
"""STFT magnitude spectrogram kernel for Trainium2 (8 NeuronCores).

Computes, for x (64, 160000):
  out[b, k, t] = |sum_n w[n] * x[b, 256*t + n] * exp(-2i*pi*k*n/1024)|
with w the normalized Hann window from the reference. Data-parallel over
batch: 8 rows per core.

Fast path vs the v1 kernel (232us):
  * Reflection fold: pair n <-> 1023-n.  The window is exactly symmetric
    (w[n] == w[1023-n] for any clamped wl / win_pow), so with
    s[q] = x[n]+x[1023-n], d[q] = x[n]-x[1023-n] (q = 0..511):
      |X[k]| = sqrt((sum_q w_q s_q cos(2pi k (q+.5)/1024))^2
             + (sum_q w_q d_q sin(2pi k (q+.5)/1024))^2)
    i.e. HALF the matmul instructions of the naive 1024-point DFT.
  * bf16 everywhere off the PE accumulators: halves DMA, enables DVE
    2x/4x modes and XBAR DMA-transpose stream loading (no PE transposes).
  * Streams S_h[p,u] = x[256u+128h+p] AND their partition-reversed
    counterparts R (from a host-side block-flipped copy of x) are loaded
    straight from DRAM with DMA transpose - the PE runs only the 512
    folded-DFT matmuls.
  * Magnitude split across engines: one square on Scalar (PSUM direct),
    the other on Vector (copy + self-mult in bf16 4x mode), sum on
    Vector, sqrt on Scalar.
"""

import sys

sys.path.insert(0, "/opt/trn_rl_repo")

import numpy as np

N = 1024
STRIDE = 256
B = 64
L = 160000
T = 622          # frames
F = 513          # rfft bins
NCORES = 8
BPC = B // NCORES  # batch rows per core
NQ = 512           # folded contraction length
C4 = 4             # folded 128-chunks
NU = 640           # padded stream columns (625 used)
LPAD = NU * 256    # padded sample count (163840)
TSPLIT = (312, 310)  # frame tile split (4B-aligned slice starts for DVE 2x)
KROWS = 512        # device freq rows; Nyquist k=512 done on host

# chunk c of s/d: first operand S_h[:, j+t], second R_h2[:, j2+t]
SMAP = {0: (0, 0), 1: (1, 0), 2: (0, 1), 3: (1, 1)}
RMAP = {0: (1, 3), 1: (0, 3), 2: (1, 2), 3: (0, 2)}

_prog_cache = {}


def _patch_fast_compile():
    """Disable the BIR simulator inside walrus codegen: it is only a
    verification aid and costs ~50 min on this kernel (vs ~3 min off)."""
    import concourse.bass_utils as bu

    if getattr(bu, "_fast_compile_patched", False):
        return
    from pathlib import Path

    from concourse.aot_env import aot_getenv

    def bir_verify_and_optimise(
        tmpdir, inp="bir.json", outp="file.neff", arch=None, *, dve_root=None
    ):
        cmd = [
            bu.get_walrus_driver(),
            "--pass",
            ",".join(
                [
                    "birverifier",
                    "runtime_memory_reservation",
                    "lower_act",
                    "lower_dve",
                    "lower_ap_offset",
                    "codegen",
                    "neff_packager",
                ]
            ),
            "-i", inp,
            "--neff-output-filename", outp,
            "--enable-birsim=false",
            "--mem-mode=physical",
            "--policy=0",
            "--enable-ldw-opt=false",
            "--assign-static-dmas-to-sp=false",
            f"--dram-page-size={aot_getenv('NEURON_SCRATCHPAD_PAGE_SIZE', '256')}",
            "--enable-neff-debug-info=true",
            "--jobs", "8",
            *bu.get_walrus_args(
                bu.get_bir_arch(tmpdir, inp) if arch is None else arch,
                tmpdir,
                dve_root=dve_root,
            ),
        ]
        result = bu.run_command(cmd, cwd=tmpdir)
        if result is not None:
            (Path(tmpdir) / "log.txt").write_text(result.stdout)
        return f"{tmpdir}/{outp}"

    bu.bir_verify_and_optimise = bir_verify_and_optimise
    bu._fast_compile_patched = True


def _build_program():
    _patch_fast_compile()
    import concourse.bass as bass
    import concourse.mybir as mybir
    import concourse.tile as tile
    from concourse import bacc

    f32 = mybir.dt.float32
    bf16 = mybir.dt.bfloat16

    nc = bacc.Bacc("TRN2", target_bir_lowering=False, enable_partition_id=False)

    xs = nc.dram_tensor("xs", [BPC, LPAD], bf16, kind="ExternalInput")
    xr = nc.dram_tensor("xr", [BPC, LPAD], bf16, kind="ExternalInput")
    cw = nc.dram_tensor("cw", [128, C4, KROWS], bf16, kind="ExternalInput")
    sw = nc.dram_tensor("sw", [128, C4, KROWS], bf16, kind="ExternalInput")
    out = nc.dram_tensor("out", [BPC, KROWS, T], bf16, kind="ExternalOutput")

    Square = mybir.ActivationFunctionType.Square
    Sqrt = mybir.ActivationFunctionType.Sqrt

    with tile.TileContext(nc) as tc:
        with (
            tc.tile_pool(name="const", bufs=1) as const_pool,
            tc.tile_pool(name="streams", bufs=2) as st_pool,
            tc.tile_pool(name="sd", bufs=2) as sd_pool,
            tc.tile_pool(name="sq", bufs=3) as sq_pool,
            tc.tile_pool(name="outsb", bufs=3) as out_pool,
            tc.tile_pool(name="pmm", bufs=4, space="PSUM") as pmm_pool,
        ):
            cw_sb = const_pool.tile([128, C4, KROWS], bf16)
            sw_sb = const_pool.tile([128, C4, KROWS], bf16)
            nc.gpsimd.dma_start(cw_sb[:], cw.rearrange("p c k -> p c k"))
            nc.gpsimd.dma_start(sw_sb[:], sw.rearrange("p c k -> p c k"))

            # --- streams straight from DRAM via XBAR DMA transpose ---
            # The XBAR is a shared resource: concurrent transposes on two
            # queues corrupt each other's 16-row tiles, so they ALL go on
            # the sync queue (serialized).  Graduated row grouping: early
            # rows get small transposes (short pipeline fill), later rows
            # big ones (few DMAs; completion semaphores are a shared pool
    	    # and many DMAs serialize against each other).
            GROUPS = [(0, 1), (1, 1), (2, 2), (4, 4)]  # (first row, n rows)
            row_grp = {}
            st_tiles = {}
            for g, (r0, nr) in enumerate(GROUPS):
                for r in range(r0, r0 + nr):
                    row_grp[r] = (g, (r - r0) * NU)
                for kind, dram in (("s", xs), ("r", xr)):
                    vw = dram[r0 : r0 + nr].rearrange(
                        "b (u c p) -> (b u) c p", u=NU, c=2, p=128
                    )
                    for h in range(2):
                        tl = st_pool.tile(
                            [128, nr * NU], bf16, tag=f"{kind}{h}g{g}"
                        )
                        nc.sync.dma_start(tl[:], vw[:, h, :], transpose=True)
                        st_tiles[(g, kind, h)] = tl

            for b in range(BPC):
                g, base = row_grp[b]
                # --- fold: s = a + rev, d = a - rev (DVE, bf16) ---
                s_sd, d_sd = [], []
                for c in range(C4):
                    h, j = SMAP[c]
                    h2, j2 = RMAP[c]
                    av = st_tiles[(g, "s", h)][:, base + j : base + j + T]
                    bv = st_tiles[(g, "r", h2)][:, base + j2 : base + j2 + T]
                    s_c = sd_pool.tile([128, T], bf16, tag=f"s{c}")
                    d_c = sd_pool.tile([128, T], bf16, tag=f"d{c}")
                    nc.vector.tensor_add(s_c[:], av, bv)
                    nc.vector.tensor_sub(d_c[:], av, bv)
                    s_sd.append(s_c)
                    d_sd.append(d_c)

                # --- folded DFT + magnitude ---
                # GPSIMD runs NO elementwise (it shares SBUF ports with the
                # DVE and halves its throughput) - it only dispatches DMAs.
                # Flat 2D per-row tiles: 8 square slices land in ssum_row /
                # sq_v_row, ONE DMA-engine accum adds them, ONE wide sqrt
                # and ONE output DMA per row (DMA count and per-op
                # overheads stay low; 2D slice + whole-tile accum is the
                # dep-tracking pattern proven in earlier runs, 3D broke).
                ssum_row = sq_pool.tile([128, 4 * T], bf16, tag="ssum_row")
                sq_v_row = sq_pool.tile([128, 4 * T], bf16, tag="sq_v_row")
                o_row = out_pool.tile([128, 4 * T], bf16, tag="o_row")
                for f in range(4):
                    t0 = 0
                    for ti, W1 in enumerate(TSPLIT):
                        p_re = pmm_pool.tile([128, TSPLIT[0]], f32, tag="p_re")
                        p_im = pmm_pool.tile([128, TSPLIT[0]], f32, tag="p_im")
                        for c in range(C4):
                            kw = dict(start=(c == 0), stop=(c == C4 - 1))
                            nc.tensor.matmul(
                                p_re[:, 0:W1], cw_sb[:, c, 128 * f : 128 * f + 128],
                                s_sd[c][:, t0 : t0 + W1], **kw,
                            )
                            nc.tensor.matmul(
                                p_im[:, 0:W1], sw_sb[:, c, 128 * f : 128 * f + 128],
                                d_sd[c][:, t0 : t0 + W1], **kw,
                            )
                        # first square on Scalar straight into ssum_row;
                        # second square alternates Scalar / Vector
                        # (copy + self-mult) 3:5 to balance the engines
                        pa, pv = (p_im, p_re) if ti == 0 else (p_re, p_im)
                        o0 = f * T + t0
                        nc.scalar.activation(
                            ssum_row[:, o0 : o0 + W1], pa[:, 0:W1], Square
                        )
                        if (b * 8 + f * 2 + ti) % 8 < 3:
                            nc.scalar.activation(
                                sq_v_row[:, o0 : o0 + W1], pv[:, 0:W1], Square
                            )
                        else:
                            sq_c = sq_pool.tile([128, TSPLIT[0]], bf16, tag="sq_c")
                            nc.vector.tensor_copy(sq_c[:, 0:W1], pv[:, 0:W1])
                            nc.vector.tensor_mul(
                                sq_v_row[:, o0 : o0 + W1], sq_c[:, 0:W1], sq_c[:, 0:W1]
                            )
                        t0 += W1
                nc.gpsimd.dma_start(
                    ssum_row[:], sq_v_row[:], accum_op=mybir.AluOpType.add
                )
                nc.scalar.activation(o_row[:], ssum_row[:], Sqrt)
                nc.gpsimd.dma_start(
                    out[b].rearrange("(f p) t -> p f t", f=4, p=128),
                    o_row[:].rearrange("p (f t) -> p f t", f=4, t=T),
                )

    nc.compile()
    return nc


def _host_params(win_length, strides, win_pow):
    """Reproduce the reference's parameter transforms on the host."""
    wl = float(np.clip(np.asarray(win_length, np.float64)[0], N / 20.0, float(N)))
    st = float(np.clip(np.asarray(strides, np.float64)[0], 0.0, float(N)))

    es = np.full((T,), st, np.float64)
    frames = np.concatenate([[0.0], np.cumsum(es[1:])])
    idx_floor = np.floor(frames)
    idx_frac = frames - idx_floor

    if not (np.all(idx_frac == 0.0) and np.all(idx_floor == STRIDE * np.arange(T))):
        raise NotImplementedError(
            "kernel fast path requires integer frame stride of 256"
        )

    base = np.arange(N, dtype=np.float64)
    tap = 0.5 - 0.5 * np.cos(2.0 * np.pi * (base + (wl - N + 1) / 2.0) / wl)
    mask = (base >= np.ceil((N - 1 + wl) / 2.0)) | (base <= np.floor((N - 1 - wl) / 2.0))
    tap[mask] = 0.0
    tap = tap / tap.sum()
    tap = tap ** float(np.asarray(win_pow, np.float64)[0])
    # enforce the exact reflection symmetry the fold relies on
    tap = 0.5 * (tap + tap[::-1])
    return tap


def _device_weights(tap):
    """Folded half-length DFT matrices, bf16, laid out [p, c, k]."""
    import ml_dtypes

    q = np.arange(NQ, dtype=np.float64)
    k = np.arange(KROWS, dtype=np.float64)
    theta = 2.0 * np.pi * np.outer(q + 0.5, k) / N
    CW = (tap[:NQ, None] * np.cos(theta)).reshape(C4, 128, KROWS).transpose(1, 0, 2)
    SW = (tap[:NQ, None] * np.sin(theta)).reshape(C4, 128, KROWS).transpose(1, 0, 2)
    bf = ml_dtypes.bfloat16
    return (
        np.ascontiguousarray(CW.astype(bf)),
        np.ascontiguousarray(SW.astype(bf)),
    )


def _in_arrays(x):
    """Padded bf16 copy of x plus its 128-block-reversed twin."""
    import ml_dtypes

    xpad = np.zeros((B, LPAD), dtype=ml_dtypes.bfloat16)
    xpad[:, :L] = x.astype(ml_dtypes.bfloat16)
    xrev = np.ascontiguousarray(
        xpad.reshape(B, LPAD // 128, 128)[:, :, ::-1].reshape(B, LPAD)
    )
    return xpad, xrev


def kernel(x, win_length, strides, win_pow):
    from concourse.bass_utils import run_bass_kernel_spmd

    x = np.ascontiguousarray(np.asarray(x, dtype=np.float32))
    assert x.shape == (B, L)

    tap = _host_params(win_length, strides, win_pow)
    CW, SW = _device_weights(tap)
    xpad, xrev = _in_arrays(x)

    if "nc" not in _prog_cache:
        _prog_cache["nc"] = _build_program()
    nc = _prog_cache["nc"]

    in_maps = [
        {
            "xs": xpad[c * BPC : (c + 1) * BPC],
            "xr": xrev[c * BPC : (c + 1) * BPC],
            "cw": CW,
            "sw": SW,
        }
        for c in range(NCORES)
    ]
    res = run_bass_kernel_spmd(nc, in_maps, core_ids=list(range(NCORES)))
    outp = np.empty((B, F, T), dtype=np.float32)
    for c in range(NCORES):
        outp[c * BPC : (c + 1) * BPC, :KROWS, :] = res.results[c]["out"].astype(
            np.float32
        )

    # Nyquist row k=512 on host: X[512] = sum_n (-1)^n w[n] x[.,256t+n]
    wn = (tap * ((-1.0) ** np.arange(N))).astype(np.float32)
    frames_v = np.lib.stride_tricks.as_strided(
        x,
        shape=(B, T, N),
        strides=(x.strides[0], STRIDE * x.itemsize, x.itemsize),
    )
    outp[:, 512, :] = np.abs(frames_v @ wn)
    return outp


# revision 20
# speedup vs baseline: 1.3171x; 1.3171x over previous
"""STFT magnitude spectrogram kernel for Trainium2 (8 NeuronCores).

Computes, for x (64, 160000):
  out[b, k, t] = |sum_n w[n] * x[b, 256*t + n] * exp(-2i*pi*k*n/1024)|
with w the normalized Hann window from the reference. Data-parallel over
batch: 8 rows per core.

Fast path vs the v1 kernel (232us):
  * Reflection fold: pair n <-> 1023-n.  The window is exactly symmetric
    (w[n] == w[1023-n] for any clamped wl / win_pow), so with
    s[q] = x[n]+x[1023-n], d[q] = x[n]-x[1023-n] (q = 0..511):
      |X[k]| = sqrt((sum_q w_q s_q cos(2pi k (q+.5)/1024))^2
             + (sum_q w_q d_q sin(2pi k (q+.5)/1024))^2)
    i.e. HALF the matmul instructions of the naive 1024-point DFT.
  * bf16 everywhere off the PE accumulators: halves DMA, enables DVE
    2x/4x modes and XBAR DMA-transpose stream loading (no PE transposes).
  * Streams S_h[p,u] = x[256u+128h+p] AND their partition-reversed
    counterparts R (from a host-side block-flipped copy of x) are loaded
    straight from DRAM with DMA transpose - the PE runs only the 512
    folded-DFT matmuls.
  * Magnitude split across engines: one square on Scalar (PSUM direct),
    the other on Vector (copy + self-mult in bf16 4x mode), sum on
    Vector, sqrt on Scalar.
"""

import sys

sys.path.insert(0, "/opt/trn_rl_repo")

import numpy as np

N = 1024
STRIDE = 256
B = 64
L = 160000
T = 622          # frames
F = 513          # rfft bins
NCORES = 8
BPC = B // NCORES  # batch rows per core
NQ = 512           # folded contraction length
C4 = 4             # folded 128-chunks
NU = 640           # padded stream columns (625 used)
LPAD = NU * 256    # padded sample count (163840)
TSPLIT = (312, 310)  # frame tile split (4B-aligned slice starts for DVE 2x)
KROWS = 512        # device freq rows; Nyquist k=512 done on host

# chunk c of s/d: first operand S_h[:, j+t], second R_h2[:, j2+t]
SMAP = {0: (0, 0), 1: (1, 0), 2: (0, 1), 3: (1, 1)}
RMAP = {0: (1, 3), 1: (0, 3), 2: (1, 2), 3: (0, 2)}

_prog_cache = {}


def _patch_fast_compile():
    """Disable the BIR simulator inside walrus codegen: it is only a
    verification aid and costs ~50 min on this kernel (vs ~3 min off)."""
    import concourse.bass_utils as bu

    if getattr(bu, "_fast_compile_patched", False):
        return
    from pathlib import Path

    from concourse.aot_env import aot_getenv

    def bir_verify_and_optimise(
        tmpdir, inp="bir.json", outp="file.neff", arch=None, *, dve_root=None
    ):
        cmd = [
            bu.get_walrus_driver(),
            "--pass",
            ",".join(
                [
                    "birverifier",
                    "runtime_memory_reservation",
                    "lower_act",
                    "lower_dve",
                    "lower_ap_offset",
                    "codegen",
                    "neff_packager",
                ]
            ),
            "-i", inp,
            "--neff-output-filename", outp,
            "--enable-birsim=false",
            "--mem-mode=physical",
            "--policy=0",
            "--enable-ldw-opt=false",
            "--assign-static-dmas-to-sp=false",
            f"--dram-page-size={aot_getenv('NEURON_SCRATCHPAD_PAGE_SIZE', '256')}",
            "--enable-neff-debug-info=true",
            "--jobs", "8",
            *bu.get_walrus_args(
                bu.get_bir_arch(tmpdir, inp) if arch is None else arch,
                tmpdir,
                dve_root=dve_root,
            ),
        ]
        result = bu.run_command(cmd, cwd=tmpdir)
        if result is not None:
            (Path(tmpdir) / "log.txt").write_text(result.stdout)
        return f"{tmpdir}/{outp}"

    bu.bir_verify_and_optimise = bir_verify_and_optimise
    bu._fast_compile_patched = True


def _build_program():
    _patch_fast_compile()
    import concourse.bass as bass
    import concourse.mybir as mybir
    import concourse.tile as tile
    from concourse import bacc

    f32 = mybir.dt.float32
    bf16 = mybir.dt.bfloat16

    nc = bacc.Bacc("TRN2", target_bir_lowering=False, enable_partition_id=False)

    xs = nc.dram_tensor("xs", [BPC, LPAD], bf16, kind="ExternalInput")
    xr = nc.dram_tensor("xr", [BPC, LPAD], bf16, kind="ExternalInput")
    cw = nc.dram_tensor("cw", [128, C4, KROWS], bf16, kind="ExternalInput")
    sw = nc.dram_tensor("sw", [128, C4, KROWS], bf16, kind="ExternalInput")
    out = nc.dram_tensor("out", [BPC, KROWS, T], bf16, kind="ExternalOutput")

    Square = mybir.ActivationFunctionType.Square
    Sqrt = mybir.ActivationFunctionType.Sqrt

    with tile.TileContext(nc) as tc:
        with (
            tc.tile_pool(name="const", bufs=1) as const_pool,
            tc.tile_pool(name="streams", bufs=2) as st_pool,
            tc.tile_pool(name="sd", bufs=2) as sd_pool,
            tc.tile_pool(name="sq", bufs=3) as sq_pool,
            tc.tile_pool(name="outsb", bufs=3) as out_pool,
            tc.tile_pool(name="pmm", bufs=4, space="PSUM") as pmm_pool,
        ):
            cw_sb = const_pool.tile([128, C4, KROWS], bf16)
            sw_sb = const_pool.tile([128, C4, KROWS], bf16)
            nc.gpsimd.dma_start(cw_sb[:], cw.rearrange("p c k -> p c k"))
            nc.gpsimd.dma_start(sw_sb[:], sw.rearrange("p c k -> p c k"))

            # --- streams straight from DRAM via XBAR DMA transpose ---
            # The XBAR is a shared resource: concurrent transposes on two
            # queues corrupt each other's 16-row tiles, so they ALL go on
            # the sync queue (serialized).  Graduated row grouping: early
            # rows get small transposes (short pipeline fill), later rows
            # big ones (few DMAs; completion semaphores are a shared pool
    	    # and many DMAs serialize against each other).
            GROUPS = [(0, 1), (1, 1), (2, 2), (4, 4)]  # (first row, n rows)
            row_grp = {}
            st_tiles = {}
            for g, (r0, nr) in enumerate(GROUPS):
                for r in range(r0, r0 + nr):
                    row_grp[r] = (g, (r - r0) * NU)
                for kind, dram in (("s", xs), ("r", xr)):
                    vw = dram[r0 : r0 + nr].rearrange(
                        "b (u c p) -> (b u) c p", u=NU, c=2, p=128
                    )
                    for h in range(2):
                        tl = st_pool.tile(
                            [128, nr * NU], bf16, tag=f"{kind}{h}g{g}"
                        )
                        nc.sync.dma_start(tl[:], vw[:, h, :], transpose=True)
                        st_tiles[(g, kind, h)] = tl

            for b in range(BPC):
                g, base = row_grp[b]
                # --- fold: s = a + rev, d = a - rev (DVE, bf16) ---
                s_sd, d_sd = [], []
                for c in range(C4):
                    h, j = SMAP[c]
                    h2, j2 = RMAP[c]
                    av = st_tiles[(g, "s", h)][:, base + j : base + j + T]
                    bv = st_tiles[(g, "r", h2)][:, base + j2 : base + j2 + T]
                    s_c = sd_pool.tile([128, T], bf16, tag=f"s{c}")
                    d_c = sd_pool.tile([128, T], bf16, tag=f"d{c}")
                    nc.vector.tensor_add(s_c[:], av, bv)
                    nc.vector.tensor_sub(d_c[:], av, bv)
                    s_sd.append(s_c)
                    d_sd.append(d_c)

                # --- folded DFT + magnitude ---
                # GPSIMD runs NO elementwise (it shares SBUF ports with the
                # DVE and halves its throughput) - it only dispatches DMAs.
                o_all = out_pool.tile([128, 4, T], bf16, tag="o_all")
                for f in range(4):
                    ssum = sq_pool.tile([128, T], bf16, tag="ssum")
                    sq_v = sq_pool.tile([128, T], bf16, tag="sq_v")
                    t0 = 0
                    for ti, W1 in enumerate(TSPLIT):
                        p_re = pmm_pool.tile([128, TSPLIT[0]], f32, tag="p_re")
                        p_im = pmm_pool.tile([128, TSPLIT[0]], f32, tag="p_im")
                        for c in range(C4):
                            kw = dict(start=(c == 0), stop=(c == C4 - 1))
                            nc.tensor.matmul(
                                p_re[:, 0:W1], cw_sb[:, c, 128 * f : 128 * f + 128],
                                s_sd[c][:, t0 : t0 + W1], **kw,
                            )
                            nc.tensor.matmul(
                                p_im[:, 0:W1], sw_sb[:, c, 128 * f : 128 * f + 128],
                                d_sd[c][:, t0 : t0 + W1], **kw,
                            )
                        pa, pv = (p_im, p_re) if ti == 0 else (p_re, p_im)
                        nc.scalar.activation(
                            ssum[:, t0 : t0 + W1], pa[:, 0:W1], Square
                        )
                        if (b * 8 + f * 2 + ti) % 8 < 3:
                            nc.scalar.activation(
                                sq_v[:, t0 : t0 + W1], pv[:, 0:W1], Square
                            )
                        else:
                            sq_c = sq_pool.tile([128, TSPLIT[0]], bf16, tag="sq_c")
                            nc.vector.tensor_copy(sq_c[:, 0:W1], pv[:, 0:W1])
                            nc.vector.tensor_mul(
                                sq_v[:, t0 : t0 + W1], sq_c[:, 0:W1], sq_c[:, 0:W1]
                            )
                        t0 += W1
                    nc.vector.tensor_add(ssum[:], ssum[:], sq_v[:])
                    nc.scalar.activation(o_all[:, f, :], ssum[:], Sqrt)
                nc.gpsimd.dma_start(
                    out[b].rearrange("(f p) t -> p f t", f=4, p=128), o_all[:]
                )

    nc.compile()
    return nc


def _host_params(win_length, strides, win_pow):
    """Reproduce the reference's parameter transforms on the host."""
    wl = float(np.clip(np.asarray(win_length, np.float64)[0], N / 20.0, float(N)))
    st = float(np.clip(np.asarray(strides, np.float64)[0], 0.0, float(N)))

    es = np.full((T,), st, np.float64)
    frames = np.concatenate([[0.0], np.cumsum(es[1:])])
    idx_floor = np.floor(frames)
    idx_frac = frames - idx_floor

    if not (np.all(idx_frac == 0.0) and np.all(idx_floor == STRIDE * np.arange(T))):
        raise NotImplementedError(
            "kernel fast path requires integer frame stride of 256"
        )

    base = np.arange(N, dtype=np.float64)
    tap = 0.5 - 0.5 * np.cos(2.0 * np.pi * (base + (wl - N + 1) / 2.0) / wl)
    mask = (base >= np.ceil((N - 1 + wl) / 2.0)) | (base <= np.floor((N - 1 - wl) / 2.0))
    tap[mask] = 0.0
    tap = tap / tap.sum()
    tap = tap ** float(np.asarray(win_pow, np.float64)[0])
    # enforce the exact reflection symmetry the fold relies on
    tap = 0.5 * (tap + tap[::-1])
    return tap


def _device_weights(tap):
    """Folded half-length DFT matrices, bf16, laid out [p, c, k]."""
    import ml_dtypes

    q = np.arange(NQ, dtype=np.float64)
    k = np.arange(KROWS, dtype=np.float64)
    theta = 2.0 * np.pi * np.outer(q + 0.5, k) / N
    CW = (tap[:NQ, None] * np.cos(theta)).reshape(C4, 128, KROWS).transpose(1, 0, 2)
    SW = (tap[:NQ, None] * np.sin(theta)).reshape(C4, 128, KROWS).transpose(1, 0, 2)
    bf = ml_dtypes.bfloat16
    return (
        np.ascontiguousarray(CW.astype(bf)),
        np.ascontiguousarray(SW.astype(bf)),
    )


def _in_arrays(x):
    """Padded bf16 copy of x plus its 128-block-reversed twin."""
    import ml_dtypes

    xpad = np.zeros((B, LPAD), dtype=ml_dtypes.bfloat16)
    xpad[:, :L] = x.astype(ml_dtypes.bfloat16)
    xrev = np.ascontiguousarray(
        xpad.reshape(B, LPAD // 128, 128)[:, :, ::-1].reshape(B, LPAD)
    )
    return xpad, xrev


def kernel(x, win_length, strides, win_pow):
    from concourse.bass_utils import run_bass_kernel_spmd

    x = np.ascontiguousarray(np.asarray(x, dtype=np.float32))
    assert x.shape == (B, L)

    tap = _host_params(win_length, strides, win_pow)
    CW, SW = _device_weights(tap)
    xpad, xrev = _in_arrays(x)

    if "nc" not in _prog_cache:
        _prog_cache["nc"] = _build_program()
    nc = _prog_cache["nc"]

    in_maps = [
        {
            "xs": xpad[c * BPC : (c + 1) * BPC],
            "xr": xrev[c * BPC : (c + 1) * BPC],
            "cw": CW,
            "sw": SW,
        }
        for c in range(NCORES)
    ]
    res = run_bass_kernel_spmd(nc, in_maps, core_ids=list(range(NCORES)))
    outp = np.empty((B, F, T), dtype=np.float32)
    for c in range(NCORES):
        outp[c * BPC : (c + 1) * BPC, :KROWS, :] = res.results[c]["out"].astype(
            np.float32
        )

    # Nyquist row k=512 on host: X[512] = sum_n (-1)^n w[n] x[.,256t+n]
    wn = (tap * ((-1.0) ** np.arange(N))).astype(np.float32)
    frames_v = np.lib.stride_tricks.as_strided(
        x,
        shape=(B, T, N),
        strides=(x.strides[0], STRIDE * x.itemsize, x.itemsize),
    )
    outp[:, 512, :] = np.abs(frames_v @ wn)
    return outp
